# revision 10
# baseline (speedup 1.0000x reference)
"""Trainium2 Bass kernel for nn_ComplexHoloLinear.

Computes out = x @ Wr.T + cos(phase)[batch] * (x @ Wi.T) where Wr/Wi are
dense [4096, 4096] matrices assembled from COO duplicates (host-folded).

Distribution: output-feature sharding. Each of the 8 cores owns 512 output
rows; it streams its Wr.T/Wi.T slices into SBUF (fp16, chunk-interleaved so
each feature chunk is one ring-local DMA), computes cos(phase) on-device,
then per batch builds the combined weight W_b = Wr + cos_b * Wi in SBUF
(double-buffered) and streams all 8192 tokens of xT through the PE,
PSUM-accumulating over the feature chunks.

Mixed precision: feature chunks 0..25 run fp16 matmuls ([128]-deep each);
chunks 26..31 run as 3 fp8e4 DoubleRow matmuls ([256]-deep each at the
same instruction cost), cutting PE time ~9%. End-to-end rel err ~1.6e-2
(vs 4e-4 all-fp16), inside the 2e-2 budget.

Startup: the first TWO token groups' xT tiles are resident, and the first
sweep processes both groups chunk-by-chunk (8 matmuls per W chunk) so PE
consumption stays behind DMA delivery while the whole W load streams in.
The last group runs token-tile-outer from a resident buffer so the
eviction tail is one tile deep. The resident buffers double as batch-1's
combined-W storage (WAR-ordered by the tile framework).

Host side: transposes/pre-tiles x (fp16 + DoubleRow-packed fp8 tail),
scatter-adds the COO edge list into the dense per-core W.T slices, and
upcasts the fp16 output to f32.
"""

import math
from contextlib import ExitStack

import numpy as np

import concourse.bass as bass
import concourse.tile as tile
from concourse import bacc, mybir

F32 = mybir.dt.float32
F16 = mybir.dt.float16
F8 = mybir.dt.float8e4
ADD = mybir.AluOpType.add
DR = mybir.MatmulPerfMode.DoubleRow


class Cfg:
    """Full-size problem config."""

    NCORES = 8
    NTOK = 8192       # B * S tokens
    NBATCH = 4        # batches (distinct cos factors)
    F = 4096          # in features (contraction)
    RTOT = 4096       # out features
    TOKG = 512        # tokens per matmul sweep group (psum tiles of 128)
    NK8 = 6           # trailing feature chunks in fp8 (must be even)

    @property
    def RSH(self):    # rows per core
        return self.RTOT // self.NCORES

    @property
    def NK(self):     # feature chunks of 128
        return self.F // 128

    @property
    def NK16(self):   # fp16 feature chunks
        return self.NK - self.NK8

    @property
    def NP8(self):    # fp8 DoubleRow chunk-pairs
        return self.NK8 // 2

    @property
    def NTG(self):    # token groups
        return self.NTOK // self.TOKG

    @property
    def WFREE(self):  # fp16 W tile free size (all chunks, r+i interleaved)
        return self.NK * self.RSH


def build_body(ctx: ExitStack, tc: tile.TileContext, cfg: Cfg, aps: dict):
    nc = tc.nc
    xT = aps["xT"]          # [NK16*NTG*128, TOKG] fp16 pre-tiled
    xT8 = aps["xT8"]        # [NP8*NTG*128, 2*TOKG] fp8 DR-packed
    wri = aps["wri"]        # [128, 2*WFREE] fp16: per chunk k [WR_k | WI_k]
    phase = aps["phase"]    # [1, NBATCH]
    out = aps["out"]        # [NTOK, RSH] fp16

    RSH, NK, NB = cfg.RSH, cfg.NK, cfg.NBATCH
    NK16, NP8 = cfg.NK16, cfg.NP8
    TPG = cfg.TOKG // 128   # psum tiles per token group
    W16F = NK16 * RSH       # fp16 part of a combined-W buffer

    wpool = ctx.enter_context(tc.tile_pool(name="w", bufs=1))
    xpool = ctx.enter_context(tc.tile_pool(name="x", bufs=16))
    x8pool = ctx.enter_context(tc.tile_pool(name="x8", bufs=4))
    tpool = ctx.enter_context(tc.tile_pool(name="tmp", bufs=3))
    spool = ctx.enter_context(tc.tile_pool(name="stage", bufs=3))
    mpool = ctx.enter_context(tc.tile_pool(name="misc", bufs=1))
    pspool = ctx.enter_context(tc.tile_pool(name="ps", bufs=2, space="PSUM"))

    # --- cos(phase) on device: fold phase+pi/2 into [-pi, pi], then Sin LUT.
    ph = mpool.tile([128, NB], F32)
    nc.sync.dma_start(out=ph[:], in_=phase[:1, :].to_broadcast([128, NB]))
    q = mpool.tile([128, NB], F32)
    nc.vector.tensor_scalar_add(q[:], ph[:], math.pi / 2)
    msk = mpool.tile([128, NB], F32)
    nc.vector.tensor_scalar(
        out=msk[:], in0=q[:], scalar1=math.pi, scalar2=2 * math.pi,
        op0=mybir.AluOpType.is_gt, op1=mybir.AluOpType.mult,
    )
    nc.vector.tensor_tensor(out=q[:], in0=q[:], in1=msk[:],
                            op=mybir.AluOpType.subtract)
    cos_t = mpool.tile([128, NB], F32)
    nc.scalar.activation(cos_t[:], q[:], mybir.ActivationFunctionType.Sin)

    # --- stream W and the first two token groups' xT, one ring-local slab
    # per feature chunk (rings rotate per chunk): skew between rings never
    # splits a chunk. Chunk 0's pieces go to three different rings so the
    # very first combine + matmuls start as early as possible.
    W2 = wpool.tile([128, 2 * cfg.WFREE], F16)
    xbig1 = wpool.tile([128, W16F], F16, name="xbig1")
    xbig2 = wpool.tile([128, W16F], F16, name="wb1x")
    xbig1_8 = wpool.tile([128, NP8, 2, cfg.TOKG], F8, name="xbig1f8")
    xbig2_8 = wpool.tile([128, NP8, 2, cfg.TOKG], F8, name="wb1xf8")
    rings = (nc.scalar, nc.sync, nc.gpsimd)
    for k in range(NK):
        wsl = slice(k * 2 * RSH, (k + 1) * 2 * RSH)
        if k == 0:
            rs = (rings[0], rings[1], rings[2])
        else:
            rs = (rings[k % 3],) * 3
        rs[0].dma_start(out=W2[:, wsl], in_=wri[:, wsl])
        if k < NK16:
            xsl = slice(k * cfg.TOKG, (k + 1) * cfg.TOKG)
            row0 = (k * cfg.NTG + 0) * 128
            rs[1].dma_start(out=xbig1[:, xsl], in_=xT[row0:row0 + 128, :])
            row1 = (k * cfg.NTG + 1) * 128
            rs[2].dma_start(out=xbig2[:, xsl], in_=xT[row1:row1 + 128, :])
    for j in range(NP8):
        for gt, xb8 in ((0, xbig1_8), (1, xbig2_8)):
            row0 = (j * cfg.NTG + gt) * 128
            rings[(j + gt) % 3].dma_start(
                out=xb8[:, j, :, :],
                in_=xT8[row0:row0 + 128, :].rearrange("p (a c) -> p a c", a=2))

    # --- per batch: build W_b (double-buffered fp16 part + fp8 DR part),
    # then matmul all its tokens
    WB0 = wpool.tile([128, W16F], F16, name="wb0")
    WB0_8 = wpool.tile([128, NP8, 2, RSH], F8, name="wb0f8")
    ntg_per_b = cfg.NTG // NB
    last_gt = cfg.NTG - 1
    for b in range(NB):
        if b % 2 == 0:
            WB, WB8 = WB0, WB0_8
        else:
            # reuse xbig2's SBUF; WAR deps delay the write past the paired
            # sweep's reads (b=1) / batch-1 matmul reads (b=3).
            WB = wpool.tile([128, W16F], F16, name="wb1x")
            WB8 = wpool.tile([128, NP8, 2, RSH], F8, name="wb1xf8")
        for k in range(NK):
            wr_sl = slice(k * 2 * RSH, k * 2 * RSH + RSH)
            wi_sl = slice(k * 2 * RSH + RSH, (k + 1) * 2 * RSH)
            tmp = tpool.tile([128, RSH], F16)
            nc.vector.tensor_scalar(out=tmp[:], in0=W2[:, wi_sl],
                                    scalar1=cos_t[:, b:b + 1], scalar2=None,
                                    op0=mybir.AluOpType.mult)
            if k < NK16:
                dst = WB[:, k * RSH:(k + 1) * RSH]
            else:
                j, a = divmod(k - NK16, 2)
                dst = WB8[:, j, a, :]
            nc.vector.tensor_tensor(out=dst, in0=W2[:, wr_sl],
                                    in1=tmp[:], op=ADD)

        for tg in range(ntg_per_b):
            gt = b * ntg_per_b + tg
            if gt == 0:
                # paired sweep: tg0 + tg1 from resident buffers, 8 matmuls
                # per W chunk so consumption trails DMA delivery.
                pts0 = [pspool.tile([128, RSH], F32, space="PSUM",
                                    tag=f"ps{t}", name=f"ps{t}")
                        for t in range(TPG)]
                pts1 = [pspool.tile([128, RSH], F32, space="PSUM",
                                    tag=f"ps{t}", name=f"ps{t}")
                        for t in range(TPG)]
                for k in range(NK16):
                    rhs = WB[:, k * RSH:(k + 1) * RSH]
                    for pts, xb in ((pts0, xbig1), (pts1, xbig2)):
                        for t in range(TPG):
                            c0 = k * cfg.TOKG + t * 128
                            nc.tensor.matmul(
                                out=pts[t][:], lhsT=xb[:, c0:c0 + 128],
                                rhs=rhs, start=(k == 0), stop=False,
                            )
                for j in range(NP8):
                    rhs8 = WB8[:, j, :, :]
                    for pts, xb8 in ((pts0, xbig1_8), (pts1, xbig2_8)):
                        for t in range(TPG):
                            nc.tensor.matmul(
                                out=pts[t][:],
                                lhsT=xb8[:, j, :, t * 128:(t + 1) * 128],
                                rhs=rhs8, start=False, stop=(j == NP8 - 1),
                                perf_mode=DR,
                            )
                for grp, tok_base in ((pts0, 0), (pts1, cfg.TOKG)):
                    for t in range(TPG):
                        stg = spool.tile([128, RSH], F16)
                        nc.scalar.copy(out=stg[:], in_=grp[t][:])
                        tok0 = tok_base + t * 128
                        nc.gpsimd.dma_start(out=out[tok0:tok0 + 128, :],
                                            in_=stg[:])
            elif gt == 1:
                continue  # handled by the paired sweep
            elif gt == last_gt:
                # last sweep: xT resident (reuses xbig1), token-tile-outer
                # so evictions stagger and the tail is one tile deep.
                xl = wpool.tile([128, W16F], F16, name="xbig1")
                xl8 = wpool.tile([128, NP8, 2, cfg.TOKG], F8, name="xbig1f8")
                for k in range(NK16):
                    row0 = (k * cfg.NTG + gt) * 128
                    rings[k % 3].dma_start(
                        out=xl[:, k * cfg.TOKG:(k + 1) * cfg.TOKG],
                        in_=xT[row0:row0 + 128, :])
                for j in range(NP8):
                    row0 = (j * cfg.NTG + gt) * 128
                    rings[j % 3].dma_start(
                        out=xl8[:, j, :, :],
                        in_=xT8[row0:row0 + 128, :].rearrange(
                            "p (a c) -> p a c", a=2))
                ev_rings = (nc.sync, nc.scalar, nc.gpsimd, nc.sync)
                for t in range(TPG):
                    ps = pspool.tile([128, RSH], F32, space="PSUM",
                                     tag=f"ps{t}", name=f"ps{t}")
                    for k in range(NK16):
                        c0 = k * cfg.TOKG + t * 128
                        nc.tensor.matmul(
                            out=ps[:], lhsT=xl[:, c0:c0 + 128],
                            rhs=WB[:, k * RSH:(k + 1) * RSH],
                            start=(k == 0), stop=False,
                        )
                    for j in range(NP8):
                        nc.tensor.matmul(
                            out=ps[:],
                            lhsT=xl8[:, j, :, t * 128:(t + 1) * 128],
                            rhs=WB8[:, j, :, :], start=False,
                            stop=(j == NP8 - 1), perf_mode=DR,
                        )
                    stg = spool.tile([128, RSH], F16)
                    nc.scalar.copy(out=stg[:], in_=ps[:])
                    tok0 = gt * cfg.TOKG + t * 128
                    ev_rings[t].dma_start(out=out[tok0:tok0 + 128, :],
                                          in_=stg[:])
            else:
                pts = [pspool.tile([128, RSH], F32, space="PSUM",
                                   tag=f"ps{t}", name=f"ps{t}")
                       for t in range(TPG)]
                for k in range(NK16):
                    xt = xpool.tile([128, cfg.TOKG], F16)
                    row0 = (k * cfg.NTG + gt) * 128
                    rings[k % 3].dma_start(out=xt[:], in_=xT[row0:row0 + 128, :])
                    for t in range(TPG):
                        nc.tensor.matmul(
                            out=pts[t][:],
                            lhsT=xt[:, t * 128:(t + 1) * 128],
                            rhs=WB[:, k * RSH:(k + 1) * RSH],
                            start=(k == 0), stop=False,
                        )
                for j in range(NP8):
                    x8t = x8pool.tile([128, 2, cfg.TOKG], F8)
                    row0 = (j * cfg.NTG + gt) * 128
                    rings[j % 3].dma_start(
                        out=x8t[:],
                        in_=xT8[row0:row0 + 128, :].rearrange(
                            "p (a c) -> p a c", a=2))
                    for t in range(TPG):
                        nc.tensor.matmul(
                            out=pts[t][:],
                            lhsT=x8t[:, :, t * 128:(t + 1) * 128],
                            rhs=WB8[:, j, :, :], start=False,
                            stop=(j == NP8 - 1), perf_mode=DR,
                        )
                for t in range(TPG):
                    stg = spool.tile([128, RSH], F16)
                    nc.scalar.copy(out=stg[:], in_=pts[t][:])
                    tok0 = gt * cfg.TOKG + t * 128
                    nc.gpsimd.dma_start(out=out[tok0:tok0 + 128, :],
                                        in_=stg[:])


def build_nc(cfg: Cfg):
    nc = bacc.Bacc("TRN2", target_bir_lowering=False, debug=False,
                   num_devices=cfg.NCORES)
    aps = {
        # xT pre-tiled on host: row block (k*NTG + gt)*128 holds the
        # [128 feat, TOKG tok] tile for fp16 feature-chunk k, token-group gt.
        "xT": nc.dram_tensor("xT", [cfg.NK16 * cfg.NTG * 128, cfg.TOKG], F16,
                             kind="ExternalInput").ap(),
        # fp8 DR-packed: row block (j*NTG + gt)*128 holds [128 feat-low,
        # (2 ktile, TOKG tok)] for chunk-pair j (= chunks NK16+2j, NK16+2j+1).
        "xT8": nc.dram_tensor("xT8", [cfg.NP8 * cfg.NTG * 128, 2 * cfg.TOKG],
                              F8, kind="ExternalInput").ap(),
        "wri": nc.dram_tensor("wri", [128, 2 * cfg.WFREE], F16,
                              kind="ExternalInput").ap(),
        "phase": nc.dram_tensor("phase", [1, cfg.NBATCH], F32,
                                kind="ExternalInput").ap(),
        "out": nc.dram_tensor("out", [cfg.NTOK, cfg.RSH], F16,
                              kind="ExternalOutput").ap(),
    }
    with tile.TileContext(nc) as tc:
        with ExitStack() as ctx:
            build_body(ctx, tc, cfg, aps)
    nc.compile()
    return nc


def host_prep(cfg: Cfg, x, rows, cols, w_real, w_imag, phase_angles):
    """Host prep: transpose/pre-tile x (fp16 + DR-packed fp8 tail);
    scatter-add COO edges into the per-core dense W.T slices (fp16).
    Returns per-core input maps."""
    import ml_dtypes

    x = np.ascontiguousarray(np.asarray(x, dtype=np.float32)).reshape(
        cfg.NTOK, cfg.F)
    xTf = x.T  # [F, NTOK] f32 view
    C16 = cfg.NK16 * 128
    xT = np.ascontiguousarray(
        xTf[:C16].reshape(cfg.NK16, 128, cfg.NTG, cfg.TOKG)
        .transpose(0, 2, 1, 3)
    ).reshape(cfg.NK16 * cfg.NTG * 128, cfg.TOKG).astype(np.float16)
    # fp8 tail, DoubleRow packing: block (j, gt) = [128 p, 2 a, TOKG] where
    # feature = (NK16 + 2j + a)*128 + p
    x8 = np.ascontiguousarray(
        xTf[C16:].reshape(cfg.NP8, 2, 128, cfg.NTG, cfg.TOKG)
        .transpose(0, 3, 2, 1, 4)
    ).reshape(cfg.NP8 * cfg.NTG * 128, 2 * cfg.TOKG)
    xT8 = x8.astype(ml_dtypes.float8_e4m3fn)

    rows = np.asarray(rows).astype(np.int64, copy=False)
    cols = np.asarray(cols).astype(np.int64, copy=False)

    Wr = np.zeros((cfg.RTOT, cfg.F), np.float32)
    Wi = np.zeros((cfg.RTOT, cfg.F), np.float32)
    np.add.at(Wr, (rows, cols), np.asarray(w_real, np.float32))
    np.add.at(Wi, (rows, cols), np.asarray(w_imag, np.float32))

    # per-core W.T layout [128 col-partition, (col_chunk, row_in_shard)],
    # chunk-interleaved real/imag: cols [k*2*RSH, k*2*RSH+RSH) = WR chunk k.
    def relayout(W, cid):
        Wc = W[cid * cfg.RSH:(cid + 1) * cfg.RSH, :]       # [RSH, F]
        return np.ascontiguousarray(
            Wc.reshape(cfg.RSH, cfg.NK, 128).transpose(2, 1, 0)
        ).astype(np.float16)                               # [128, NK, RSH]

    phase_in = np.asarray(phase_angles, dtype=np.float32).reshape(1, cfg.NBATCH)

    in_maps = []
    for cid in range(cfg.NCORES):
        wri = np.empty((128, cfg.NK, 2, cfg.RSH), np.float16)
        wri[:, :, 0, :] = relayout(Wr, cid)
        wri[:, :, 1, :] = relayout(Wi, cid)
        in_maps.append({
            "xT": xT,
            "xT8": xT8,
            "phase": phase_in,
            "wri": wri.reshape(128, 2 * cfg.WFREE),
        })
    return in_maps


_NC_CACHE = {}
LAST_RESULTS = None  # BassKernelResults of the most recent kernel() call


def kernel(x, rows, cols, w_real, w_imag, phase_angles, out_features=4096,
           **_ignored):
    from concourse.bass_utils import run_bass_kernel_spmd

    global LAST_RESULTS
    cfg = Cfg()
    assert int(out_features) == cfg.RTOT

    if "nc" not in _NC_CACHE:
        _NC_CACHE["nc"] = build_nc(cfg)
    nc = _NC_CACHE["nc"]

    in_maps = host_prep(cfg, x, rows, cols, w_real, w_imag, phase_angles)
    res = run_bass_kernel_spmd(nc, in_maps, core_ids=list(range(cfg.NCORES)))
    LAST_RESULTS = res
    out = np.concatenate([res.results[c]["out"] for c in range(cfg.NCORES)],
                         axis=1).astype(np.float32)
    return out.reshape(cfg.NTOK // 2048, 2048, cfg.RTOT)
